# revision 1
# baseline (speedup 1.0000x reference)
"""Trainium2 Bass kernel for a classic Mamba block (B=2, L=2048, Dm=1024,
E=2048, N=16, R=64, K=3) running SPMD on 8 NeuronCores.

Sharding: tensor-parallel on inner dim E (E_loc = 256 per core).
 - input_proj column-parallel, depthwise conv + selective scan fully local,
 - W_sel partial products AllReduce'd ([96, L] per batch),
 - output_proj row-parallel with a token-dim ReduceScatter at the end.

The selective scan uses the native VectorE `tensor_tensor_scan` instruction
(state = a*state + b along the free dim) on a layout where the 128 SBUF
partitions carry (8 e-channels x 16 ssm-states) and the free dim carries
time.  The [B, E, N, L] expansions (delta*A, delta*u replicated over N) are
produced by TensorE matmuls against host-precomputed one-hot matrices, and
the C-contraction over N is likewise a TensorE matmul.
"""

import os
import sys

if "/opt/trn_rl_repo" not in sys.path:
    sys.path.insert(0, "/opt/trn_rl_repo")

import numpy as np

# ---------------------------------------------------------------------------
# Problem constants (hardcoded per contract)
B = 2
L = 2048          # sequence length per batch
DM = 1024         # model dim
E = 2048          # inner dim
N = 16            # ssm state dim
R = 64            # dt rank
K = 3             # conv kernel
N_CORES = 8
E_LOC = E // N_CORES          # 256
NS = E_LOC // 128             # e-subtiles per core (2)
NOCT = 16                     # octets per 128-partition subtile
EPB = 128 // N                # 8 e-channels per octet

FC = 512                      # psum free chunk
SCAN_DT = "bf16"              # "bf16" or "f32" scan/expansion path

_PROGRAM_CACHE = {}


def build_program(Lb=L, scan_dt=None):
    """Build + compile the 8-core SPMD Tile program. Lb = per-batch length
    (smaller for simulator tests). scan_dt: "bf16" or "f32" for the scan/
    expansion path tensors."""
    if scan_dt is None:
        scan_dt = SCAN_DT
    key = (Lb, str(scan_dt))
    if key in _PROGRAM_CACHE:
        return _PROGRAM_CACHE[key]

    import concourse.bacc as bacc
    import concourse.mybir as mybir
    import concourse.tile as tile
    import concourse.tile_utils as tile_utils

    # stale constant leaves 16KB/partition unused; raise it a bit
    if getattr(tile_utils, "max_sbuf_usage", None) is not None:
        tile_utils.max_sbuf_usage = max(tile_utils.max_sbuf_usage, 204 * 1024)

    f32 = mybir.dt.float32
    f32r = mybir.dt.float32r
    bf16 = mybir.dt.bfloat16
    f16 = mybir.dt.float16
    exp_dt = bf16 if scan_dt == "bf16" else f32r
    proj_dt = f16 if scan_dt == "bf16" else f32r
    scan_dt = bf16 if scan_dt == "bf16" else f32
    tok = B * Lb
    FH = Lb // 2                 # in-proj f-half
    FCl = min(FC, FH)            # in-proj psum free chunk
    FCs = min(FC, Lb)            # scan-phase psum free chunk (bank-sized)
    n_fc = Lb // FCs             # scan-phase chunks per batch

    nc = bacc.Bacc("TRN2", target_bir_lowering=False, debug=False,
                   num_devices=N_CORES)

    # ---------------- DRAM I/O ----------------
    xT = nc.dram_tensor("xT", [DM, tok], proj_dt, kind="ExternalInput")
    w_inT = nc.dram_tensor("w_inT", [DM, 2 * E_LOC], proj_dt, kind="ExternalInput")
    conv_w = nc.dram_tensor("conv_w", [128, NS, K], f32, kind="ExternalInput")
    conv_b = nc.dram_tensor("conv_b", [128, NS], f32, kind="ExternalInput")
    w_selT = nc.dram_tensor("w_selT", [128, NS, R + 2 * N], f32r,
                            kind="ExternalInput")
    dt_wT = nc.dram_tensor("dt_wT", [R, E_LOC], f32r, kind="ExternalInput")
    dt_b = nc.dram_tensor("dt_b", [128, NS], f32, kind="ExternalInput")
    a_scale = nc.dram_tensor("a_scale", [128, NS * NOCT], f32,
                             kind="ExternalInput")
    w_nrep = nc.dram_tensor("w_nrep", [N, 128], f32r, kind="ExternalInput")
    w_red = nc.dram_tensor("w_red", [128, NOCT * 128], exp_dt,
                       kind="ExternalInput")
    d_par = nc.dram_tensor("d_par", [128, NS], f32, kind="ExternalInput")
    w_outT = nc.dram_tensor("w_outT", [128, NS, DM], proj_dt,
                            kind="ExternalInput")

    out_loc = nc.dram_tensor("out_loc", [tok // N_CORES, DM], f32,
                             kind="ExternalOutput")

    # internal DRAM
    z_sp = nc.dram_tensor("z_sp", [E_LOC, tok], f32)
    delta_sp = nc.dram_tensor("delta_sp", [E_LOC, tok], scan_dt)
    du_sp = nc.dram_tensor("du_sp", [E_LOC, tok], scan_dt)
    u_sp = nc.dram_tensor("u_sp", [E_LOC, tok], f32r)
    ar_in = [[nc.dram_tensor(f"ar_in{b}_{h}", [R + 2 * N, Lb // 2], f32)
              for h in range(2)] for b in range(B)]
    ar_out = [[nc.dram_tensor(f"ar_out{b}_{h}", [R + 2 * N, Lb // 2], f32,
                              addr_space="Shared") for h in range(2)]
              for b in range(B)]
    part = [nc.dram_tensor(f"part{b}", [Lb, DM], f32) for b in range(B)]
    rs_out = [[nc.dram_tensor(f"rs_out{b}_{h}",
                              [Lb // 2 // N_CORES, DM], f32)
               for h in range(2)] for b in range(B)]

    rg = [list(range(N_CORES))]
    AF = mybir.ActivationFunctionType
    OP = mybir.AluOpType

    with tile.TileContext(nc) as tc:
        with tc.tile_pool(name="consts", bufs=1) as consts, \
             tc.tile_pool(name="pbig", bufs=1, space="PSUM") as pbig, \
             tc.tile_pool(name="pchunk", bufs=4, space="PSUM") as pchunk:

            # ---- load constants ----
            a_scale_sb = consts.tile([128, NS * NOCT], f32)
            nc.sync.dma_start(out=a_scale_sb[:], in_=a_scale[:])
            w_nrep_sb = consts.tile([N, 128], f32r)
            nc.sync.dma_start(out=w_nrep_sb[:], in_=w_nrep[:])
            w_red_sb = consts.tile([128, NOCT, 128], exp_dt)
            nc.sync.dma_start(out=w_red_sb[:], in_=w_red[:].rearrange(
                "p (q m) -> p q m", m=128))
            conv_w_sb = consts.tile([128, NS, K], f32)
            nc.sync.dma_start(out=conv_w_sb[:], in_=conv_w[:])
            conv_b_sb = consts.tile([128, NS], f32)
            nc.sync.dma_start(out=conv_b_sb[:], in_=conv_b[:])
            w_selT_sb = consts.tile([128, NS, R + 2 * N], f32r)
            nc.sync.dma_start(out=w_selT_sb[:], in_=w_selT[:])
            dt_wT_sb = consts.tile([R, E_LOC], f32r)
            nc.sync.dma_start(out=dt_wT_sb[:], in_=dt_wT[:])
            dt_b_sb = consts.tile([128, NS], f32)
            nc.sync.dma_start(out=dt_b_sb[:], in_=dt_b[:])
            d_par_sb = consts.tile([128, NS], f32)
            nc.sync.dma_start(out=d_par_sb[:], in_=d_par[:])
            w_outT_sb = consts.tile([128, NS, DM], proj_dt)
            nc.sync.dma_start(out=w_outT_sb[:], in_=w_outT[:])

            # ======= phase 1 per batch: in-proj, conv, dbc, AllReduce =====
            with tc.tile_pool(name="wip", bufs=1) as wip_pool, \
                 tc.tile_pool(name="xt", bufs=10) as xt_pool, \
                 tc.tile_pool(name="xc", bufs=2) as xc_pool, \
                 tc.tile_pool(name="cv", bufs=1) as cv_pool, \
                 tc.tile_pool(name="u", bufs=4) as u_pool, \
                 tc.tile_pool(name="st1", bufs=4) as st1_pool, \
                 tc.tile_pool(name="dbcp", bufs=2) as dbcp_pool:
                w_inT_sb = wip_pool.tile([128, DM // 128, 2 * E_LOC], proj_dt)
                nc.sync.dma_start(
                    out=w_inT_sb[:],
                    in_=w_inT[:].rearrange("(k p) m -> p k m", p=128))

                for b in range(B):
                    u_tiles = {}
                    xc_tiles = {}
                    for s in range(NS):
                        xc_tiles[s] = xc_pool.tile([128, Lb], f32, tag="xc", name=f"xc_{b}_{s}")
                        u_tiles[s] = u_pool.tile([128, Lb], f32r, tag="u", name=f"u_{b}_{s}")
                    for fh in range(2):
                        xt_sb = []
                        for k in range(DM // 128):
                            t = xt_pool.tile([128, FH], proj_dt, tag="xt")
                            nc.sync.dma_start(
                                out=t[:],
                                in_=xT[k * 128:(k + 1) * 128,
                                       b * Lb + fh * FH:b * Lb + (fh + 1) * FH])
                            xt_sb.append(t)

                        def mgroup(m):
                            s = m % 2
                            pcs = [pchunk.tile([128, FCl], f32, tag="pc",
                                               name=f"pc_{b}_{fh}_{m}_{i}")
                                   for i in range(FH // FCl)]
                            for k in range(DM // 128):
                                for c in range(FH // FCl):
                                    nc.tensor.matmul(
                                        pcs[c][:],
                                        lhsT=w_inT_sb[:, k,
                                                m * 128:(m + 1) * 128],
                                        rhs=xt_sb[k][:, c * FCl:(c + 1) * FCl],
                                        start=(k == 0),
                                        stop=(k == DM // 128 - 1))
                            for c in range(FH // FCl):
                                off = fh * FH + c * FCl
                                if m < 2:
                                    nc.scalar.copy(
                                        xc_tiles[s][:, off:off + FCl], pcs[c][:])
                                else:
                                    sgz = st1_pool.tile(
                                        [128, FCl], f32, tag="sg",
                                        name=f"sgz_{b}_{fh}_{m}_{c}")
                                    nc.scalar.activation(
                                        sgz[:], pcs[c][:], AF.Sigmoid)
                                    stz = st1_pool.tile(
                                        [128, FCl], f32, tag="st",
                                        name=f"stz_{b}_{fh}_{m}_{c}")
                                    nc.vector.tensor_mul(
                                        stz[:], pcs[c][:], sgz[:])
                                    nc.sync.dma_start(
                                        out=z_sp[s * 128:(s + 1) * 128,
                                                 b * Lb + off:b * Lb + off + FCl],
                                        in_=stz[:])

                        for m in range(2):   # xc halves first
                            mgroup(m)
                        # conv + silu for this half
                        lo, hi = fh * FH, (fh + 1) * FH
                        for s in range(NS):
                            xc = xc_tiles[s]
                            t1 = cv_pool.tile([128, FH], f32, tag="t1")
                            nc.vector.tensor_scalar_mul(
                                t1[:], xc[:, lo:hi], conv_w_sb[:, s, 2:3])
                            t2 = cv_pool.tile([128, FH], f32, tag="t2")
                            e1 = 1 if fh == 0 else 0
                            nc.vector.scalar_tensor_tensor(
                                t2[:, e1:FH], xc[:, lo + e1 - 1:hi - 1],
                                conv_w_sb[:, s, 1:2], t1[:, e1:FH],
                                op0=OP.mult, op1=OP.add)
                            e2 = 2 if fh == 0 else 0
                            t3 = cv_pool.tile([128, FH], f32, tag="t1")
                            nc.vector.scalar_tensor_tensor(
                                t3[:, e2:FH], xc[:, lo + e2 - 2:hi - 2],
                                conv_w_sb[:, s, 0:1], t2[:, e2:FH],
                                op0=OP.mult, op1=OP.add)
                            if fh == 0:
                                nc.vector.tensor_copy(t2[:, 0:1], t1[:, 0:1])
                                nc.vector.tensor_copy(t3[:, 0:2], t2[:, 0:2])
                            t4 = cv_pool.tile([128, FH], f32, tag="t2")
                            nc.vector.tensor_scalar_add(
                                t4[:], t3[:], conv_b_sb[:, s:s + 1])
                            sg = cv_pool.tile([128, FH], f32, tag="t1")
                            nc.scalar.activation(sg[:], t4[:], AF.Sigmoid)
                            nc.vector.tensor_mul(
                                u_tiles[s][:, lo:hi], t4[:], sg[:])
                            nc.sync.dma_start(
                                out=u_sp[s * 128:(s + 1) * 128,
                                         b * Lb + lo:b * Lb + hi],
                                in_=u_tiles[s][:, lo:hi])
                        # dbc partial for this half + AllReduce
                        dbc_part = dbcp_pool.tile([R + 2 * N, FH], f32,
                                                  tag="dbc_part",
                                                  name=f"dbcp_{b}_{fh}")
                        for c in range(FH // FCl):
                            pd = pchunk.tile([R + 2 * N, FCl], f32, tag="pc",
                                             name=f"pdb_{b}_{fh}_{c}")
                            for s in range(NS):
                                nc.tensor.matmul(
                                    pd[:],
                                    lhsT=w_selT_sb[:, s, :],
                                    rhs=u_tiles[s][:,
                                        lo + c * FCl:lo + (c + 1) * FCl],
                                    start=(s == 0), stop=(s == NS - 1))
                            nc.scalar.copy(dbc_part[:, c * FCl:(c + 1) * FCl],
                                           pd[:])
                        nc.sync.dma_start(out=ar_in[b][fh][:], in_=dbc_part[:])
                        nc.gpsimd.collective_compute(
                            "AllReduce", OP.add, replica_groups=rg,
                            ins=[ar_in[b][fh][:]], outs=[ar_out[b][fh][:]])
                        # z halves after the AR is on its way
                        for m in range(2, 4):
                            mgroup(m)

            # ======= phase 2: delta, scan, gate, out-proj (pipelined) ======
            import concourse.bass as _bass
            with tc.tile_pool(name="dbcf", bufs=1) as dbcf_pool, \
                 tc.tile_pool(name="delta", bufs=2) as delta_pool, \
                 tc.tile_pool(name="du", bufs=2) as du_pool, \
                 tc.tile_pool(name="bc", bufs=2) as bc_pool, \
                 tc.tile_pool(name="sw", bufs=12) as sw_pool, \
                 tc.tile_pool(name="y", bufs=3) as y_pool, \
                 tc.tile_pool(name="zg", bufs=1) as zg_pool, \
                 tc.tile_pool(name="st2", bufs=3) as st2_pool, \
                 tc.tile_pool(name="uld", bufs=2) as uld_pool:

                y_tiles = {}

                def prep_batch(b):
                    dtlow_sb = dbcf_pool.tile([R, Lb], f32r,
                                              tag="dtlow", name=f"dtlow{b}")
                    bt_sb = dbcf_pool.tile([N, Lb], f32r, tag="bt",
                                           name=f"bt{b}")
                    ct_sb = dbcf_pool.tile([N, Lb], f32r, tag="ct",
                                           name=f"ct{b}")
                    for dst, lo, hi, nm in ((dtlow_sb, 0, R, "dtl"),
                                            (bt_sb, R, R + N, "bt"),
                                            (ct_sb, R + N, R + 2 * N, "ct")):
                        stg = dbcf_pool.tile([R, Lb], f32, tag="arstage",
                                             name=f"arst_{b}_{nm}")
                        for h in range(2):
                            nc.sync.dma_start(
                                out=stg[0:hi - lo,
                                        h * (Lb // 2):(h + 1) * (Lb // 2)],
                                in_=ar_out[b][h][lo:hi, :])
                        nc.scalar.copy(dst[:], stg[0:hi - lo, :])
                    brep = bc_pool.tile([128, Lb], scan_dt, tag="brep",
                                        name=f"brep{b}")
                    crep = bc_pool.tile([128, Lb], scan_dt, tag="crep",
                                        name=f"crep{b}")
                    for (dst, srct) in ((brep, bt_sb), (crep, ct_sb)):
                        for c in range(n_fc):
                            pr = pchunk.tile([128, FCs], f32, tag="pc",
                                             name=f"pr_{b}_{c}")
                            nc.tensor.matmul(
                                pr[:],
                                lhsT=w_nrep_sb[:],
                                rhs=srct[:, c * FCs:(c + 1) * FCs],
                                start=True, stop=True)
                            nc.scalar.copy(dst[:, c * FCs:(c + 1) * FCs],
                                           pr[:])
                    return dtlow_sb, brep, crep

                def prep_s(b, s, dtlow_sb):
                    delta = delta_pool.tile([128, Lb], exp_dt, tag="delta",
                                            name=f"delta_{b}_{s}")
                    for c in range(n_fc):
                        pd = pchunk.tile([128, FCs], f32, tag="pc",
                                         name=f"pd_{b}_{s}_{c}")
                        nc.tensor.matmul(
                            pd[:],
                            lhsT=dt_wT_sb[:, s * 128:(s + 1) * 128],
                            rhs=dtlow_sb[:, c * FCs:(c + 1) * FCs],
                            start=True, stop=True)
                        et = st2_pool.tile(
                            [128, FCs], f32, tag="et",
                            name=f"et_{b}_{s}_{c}")
                        nc.scalar.activation(
                            et[:], pd[:], AF.Exp,
                            bias=dt_b_sb[:, s:s + 1])
                        nc.scalar.activation(
                            delta[:, c * FCs:(c + 1) * FCs], et[:],
                            AF.Ln, bias=1.0)
                    ut = uld_pool.tile([128, Lb], f32r, tag="uld",
                                       name=f"uld_{b}_{s}")
                    nc.sync.dma_start(
                        out=ut[:],
                        in_=u_sp[s * 128:(s + 1) * 128,
                                 b * Lb:(b + 1) * Lb])
                    du = du_pool.tile([128, Lb], exp_dt, tag="du",
                                      name=f"du_{b}_{s}")
                    for c in range(n_fc):
                        cs = slice(c * FCs, (c + 1) * FCs)
                        nc.vector.tensor_mul(du[:, cs], delta[:, cs],
                                             ut[:, cs])
                        nc.sync.dma_start(
                            out=delta_sp[s * 128:(s + 1) * 128,
                                         b * Lb + c * FCs:
                                         b * Lb + (c + 1) * FCs],
                            in_=delta[:, cs])
                        nc.sync.dma_start(
                            out=du_sp[s * 128:(s + 1) * 128,
                                      b * Lb + c * FCs:
                                      b * Lb + (c + 1) * FCs],
                            in_=du[:, cs])
                    return ut, delta, du

                def bcast_ap(src_t, row0, b, c):
                    sl = src_t[row0:row0 + EPB,
                               b * Lb + c * FCs:b * Lb + (c + 1) * FCs]
                    return _bass.AP(
                        tensor=sl.tensor, offset=sl.offset,
                        ap=[list(sl.ap[0]), [0, N], list(sl.ap[1])])

                def octet_loop(b, s, brep, crep, delta, du, bg=None):
                    py = pbig.tile([128, Lb], f32, tag="pbig",
                                   name=f"py_{b}_{s}")

                    def emit_yred(o, hc_t):
                        for c in range(n_fc):
                            nc.tensor.matmul(
                                py[:, c * FCs:(c + 1) * FCs],
                                lhsT=w_red_sb[:, o, :],
                                rhs=hc_t[:, c * FCs:(c + 1) * FCs],
                                start=(o == 0), stop=(o == NOCT - 1))

                    def emit_scan_hc(o, ab):
                        a_t, b_t = ab
                        h_sb = sw_pool.tile([128, Lb], scan_dt, tag="sw",
                                            name=f"h_{b}_{s}_{o}")
                        nc.vector.tensor_tensor_scan(
                            h_sb[:], a_t[:], b_t[:], 0.0,
                            op0=OP.mult, op1=OP.add)
                        hc_sb = sw_pool.tile([128, Lb], exp_dt, tag="sw",
                                             name=f"hc_{b}_{s}_{o}")
                        nc.vector.tensor_mul(hc_sb[:], h_sb[:], crep[:])
                        return hc_sb

                    prev_ab = None
                    prev_hc = None
                    for o in range(NOCT):
                        row0 = s * 128 + o * EPB
                        a_sb = sw_pool.tile([128, Lb], scan_dt, tag="sw",
                                            name=f"a_{b}_{s}_{o}")
                        for c in range(n_fc):
                            nc.sync.dma_start(
                                out=a_sb[:, c * FCs:(c + 1) * FCs],
                                in_=bcast_ap(delta_sp, row0, b, c))
                        nc.scalar.activation(
                            a_sb[:], a_sb[:], AF.Exp,
                            scale=a_scale_sb[:,
                                  s * NOCT + o:s * NOCT + o + 1])
                        b_sb = sw_pool.tile([128, Lb], scan_dt, tag="sw",
                                            name=f"b_{b}_{s}_{o}")
                        for c in range(n_fc):
                            nc.sync.dma_start(
                                out=b_sb[:, c * FCs:(c + 1) * FCs],
                                in_=bcast_ap(du_sp, row0, b, c))
                        nc.vector.tensor_mul(b_sb[:], b_sb[:], brep[:])
                        if bg is not None:
                            next(bg, None)
                        if prev_hc is not None:
                            emit_yred(*prev_hc)
                            prev_hc = None
                        if prev_ab is not None:
                            prev_hc = (o - 1, emit_scan_hc(o - 1, prev_ab))
                        prev_ab = (a_sb, b_sb)
                    prev_hc2 = (NOCT - 1, emit_scan_hc(NOCT - 1, prev_ab))
                    if prev_hc is not None:
                        emit_yred(*prev_hc)
                    emit_yred(*prev_hc2)
                    return py

                def yasm(b, s, ut, py):
                    yg = y_pool.tile([128, Lb], proj_dt, tag="y",
                                     name=f"yg_{b}_{s}")
                    for c in range(n_fc):
                        nc.vector.scalar_tensor_tensor(
                            yg[:, c * FCs:(c + 1) * FCs],
                            ut[:, c * FCs:(c + 1) * FCs],
                            d_par_sb[:, s:s + 1],
                            py[:, c * FCs:(c + 1) * FCs],
                            op0=OP.mult, op1=OP.add)
                    zt = zg_pool.tile([128, Lb], f32, tag="z",
                                      name=f"zt_{b}_{s}")
                    nc.sync.dma_start(
                        out=zt[:],
                        in_=z_sp[s * 128:(s + 1) * 128,
                                 b * Lb:(b + 1) * Lb])
                    nc.vector.tensor_mul(yg[:], yg[:], zt[:])
                    y_tiles[(b, s)] = yg

                def outproj_gen(b):
                    HB = Lb // 2 // 128          # mt tiles per half
                    HR = Lb // 2 // N_CORES      # rows per rank per half
                    for mt in range(Lb // 128):
                        for f in range(DM // FCs):
                            po = pchunk.tile([128, FCs], f32, tag="pc",
                                             name=f"po_{b}_{mt}_{f}")
                            for s in range(NS):
                                nc.tensor.matmul(
                                    po[:],
                                    lhsT=y_tiles[(b, s)][:,
                                            mt * 128:(mt + 1) * 128],
                                    rhs=w_outT_sb[:, s,
                                           f * FCs:(f + 1) * FCs],
                                    start=(s == 0), stop=(s == NS - 1))
                            sto = st2_pool.tile(
                                [128, FCs], f32, tag="st",
                                name=f"sto_{b}_{mt}_{f}")
                            nc.scalar.copy(sto[:], po[:])
                            nc.sync.dma_start(
                                out=part[b][mt * 128:(mt + 1) * 128,
                                            f * FCs:(f + 1) * FCs],
                                in_=sto[:])
                        if mt == HB - 1 or mt == 2 * HB - 1:
                            h = mt // HB
                            nc.gpsimd.collective_compute(
                                "ReduceScatter", OP.add, replica_groups=rg,
                                ins=[part[b][h * (Lb // 2):
                                             (h + 1) * (Lb // 2), :]],
                                outs=[rs_out[b][h][:]])
                            nc.gpsimd.dma_start(
                                out=out_loc[(b * 2 + h) * HR:
                                            (b * 2 + h + 1) * HR, :],
                                in_=rs_out[b][h][:])
                        yield

                # --- emission schedule ---
                dtlow0, brep0, crep0 = prep_batch(0)
                ut0, dl, duu = prep_s(0, 0, dtlow0)
                py = octet_loop(0, 0, brep0, crep0, dl, duu)
                yasm(0, 0, ut0, py)
                ut1, dl, duu = prep_s(0, 1, dtlow0)
                dtlow1, brep1, crep1 = prep_batch(1)
                py = octet_loop(0, 1, brep0, crep0, dl, duu)
                yasm(0, 1, ut1, py)
                ut2, dl, duu = prep_s(1, 0, dtlow1)
                g0 = outproj_gen(0)
                py = octet_loop(1, 0, brep1, crep1, dl, duu, bg=g0)
                for _ in g0:
                    pass
                yasm(1, 0, ut2, py)
                ut3, dl, duu = prep_s(1, 1, dtlow1)
                py = octet_loop(1, 1, brep1, crep1, dl, duu)
                yasm(1, 1, ut3, py)
                g1 = outproj_gen(1)
                for _ in g1:
                    pass

    nc.compile()
    _PROGRAM_CACHE[key] = nc
    return nc


# ---------------------------------------------------------------------------
def host_prep(inputs, Lb=L, scan_dt=None):
    """Per-core input dicts from the full problem inputs."""
    x = np.asarray(inputs["x"], np.float32)
    W_in = np.asarray(inputs["W_in"], np.float32)
    conv_w = np.asarray(inputs["conv_w"], np.float32)
    conv_b = np.asarray(inputs["conv_b"], np.float32)
    W_sel = np.asarray(inputs["W_sel"], np.float32)
    dt_w = np.asarray(inputs["dt_w"], np.float32)
    dt_b = np.asarray(inputs["dt_b"], np.float32)
    A_log = np.asarray(inputs["A_log"], np.float32)
    D_param = np.asarray(inputs["D_param"], np.float32)
    W_out = np.asarray(inputs["W_out"], np.float32)

    if scan_dt is None:
        scan_dt = SCAN_DT
    import ml_dtypes
    edt = ml_dtypes.bfloat16 if scan_dt == "bf16" else np.float32
    pdt = np.float16 if scan_dt == "bf16" else np.float32
    tok = B * Lb
    xT = np.ascontiguousarray(
        x[:, :Lb, :].reshape(tok, DM).T)            # [DM, tok]
    A = -np.exp(A_log.astype(np.float64)).astype(np.float32)   # [E, N]

    w_nrep_mat = np.zeros((N, 128), np.float32)
    w_red_mat = np.zeros((NOCT, 128, 128), np.float32)
    for o in range(NOCT):
        for j in range(EPB):
            for n in range(N):
                w_nrep_mat[n, j * N + n] = 1.0
                w_red_mat[o, j * N + n, o * EPB + j] = 1.0

    in_maps = []
    for k in range(N_CORES):
        es = slice(k * E_LOC, (k + 1) * E_LOC)
        W_in_loc = np.concatenate([W_in[k * E_LOC:(k + 1) * E_LOC],
                                   W_in[E + k * E_LOC:E + (k + 1) * E_LOC]],
                                  axis=0)            # [2*E_LOC, DM]
        A_loc = A[es]                                # [E_LOC, N]

        a_scale_mat = np.zeros((128, NS * NOCT), np.float32)
        for s in range(NS):
            for o in range(NOCT):
                for p in range(128):
                    a_scale_mat[p, s * NOCT + o] = \
                        A_loc[s * 128 + o * EPB + p // N, p % N]

        def two(v):  # [E_LOC] -> [128, NS]
            return np.ascontiguousarray(v.reshape(NS, 128).T)

        in_maps.append({
            "xT": xT.astype(pdt),
            "w_inT": np.ascontiguousarray(W_in_loc.T).astype(pdt),
            "conv_w": np.ascontiguousarray(
                conv_w[es, 0, :].reshape(NS, 128, K).transpose(1, 0, 2)),
            "conv_b": two(conv_b[es]),
            "w_selT": np.ascontiguousarray(
                W_sel[:, es].T.reshape(NS, 128, R + 2 * N).transpose(1, 0, 2)),
            "dt_wT": np.ascontiguousarray(dt_w[es].T),
            "dt_b": two(dt_b[es]),
            "a_scale": a_scale_mat,
            "w_nrep": w_nrep_mat,
            "w_red": np.ascontiguousarray(
                w_red_mat.transpose(1, 0, 2).reshape(
                    128, NOCT * 128)).astype(edt),
            "d_par": two(D_param[es]),
            "w_outT": np.ascontiguousarray(
                W_out[:, es].T.reshape(NS, 128, DM).transpose(
                    1, 0, 2)).astype(pdt),
        })
    return in_maps


def assemble_output(results, Lb=L):
    out = np.empty((B, Lb, DM), np.float32)
    hr = Lb // 2 // N_CORES
    for c in range(N_CORES):
        chunk = results[c]["out_loc"]            # [B*2*hr, DM]
        for b in range(B):
            for h in range(2):
                out[b, h * (Lb // 2) + c * hr:
                       h * (Lb // 2) + (c + 1) * hr, :] = \
                    chunk[(b * 2 + h) * hr:(b * 2 + h + 1) * hr, :]
    return out


def kernel(**inputs) -> np.ndarray:
    from concourse import bass_utils
    nc = build_program()
    in_maps = host_prep(inputs)
    res = bass_utils.run_bass_kernel_spmd(nc, in_maps, list(range(N_CORES)))
    return assemble_output(res.results).astype(np.float32)



# revision 13
# speedup vs baseline: 1.2556x; 1.2556x over previous
"""Trainium2 Bass kernel for a classic Mamba block (B=2, L=2048, Dm=1024,
E=2048, N=16, R=64, K=3) running SPMD on 8 NeuronCores.

Sharding: tensor-parallel on inner dim E (E_loc = 256 per core).

v2 architecture ("layout D"): the selective scan keeps 128 e-channels in
SBUF partitions and time in the free dim; the N=16 ssm states are processed
as 16 sequential scan tiles per (batch, e-subtile).  delta/du are consumed
directly from SBUF (no DRAM broadcast round-trip, which gated v1); only the
small [16, L] B/C rows are partition-broadcast.  The n-contraction
(y = sum_n C_n * h_n) is identity-matmul PSUM accumulation on TensorE, the
D*u skip term is a diag(D) matmul into the same PSUM bank, the causal
depthwise conv is 3 diag(w) matmuls, and silu(z) gating is fused into the
PSUM drain.  Collectives: one merged AllReduce per batch for the selective
projection, and the output-projection ReduceScatter is split into quarter
chunks issued as soon as their token blocks finish.
"""

import sys

if "/opt/trn_rl_repo" not in sys.path:
    sys.path.insert(0, "/opt/trn_rl_repo")

import numpy as np

# ---------------------------------------------------------------------------
# Problem constants (hardcoded per contract)
B = 2
L = 2048          # sequence length per batch
DM = 1024         # model dim
E = 2048          # inner dim
N = 16            # ssm state dim
R = 64            # dt rank
K = 3             # conv kernel
N_CORES = 8
E_LOC = E // N_CORES          # 256
NS = E_LOC // 128             # e-subtiles per core (2)

FC = 512                      # psum free chunk (one bank)

# knobs
GPS_HC = ()                   # n-indices whose hc-mul runs on GpSimd
GPS_BM = ()                   # n-indices whose b-mul runs on GpSimd

_PROGRAM_CACHE = {}


def build_program(Lb=L):
    key = (Lb, tuple(GPS_HC), tuple(GPS_BM))
    if key in _PROGRAM_CACHE:
        return _PROGRAM_CACHE[key]

    import concourse.bacc as bacc
    import concourse.mybir as mybir
    import concourse.tile as tile
    import concourse.tile_utils as tile_utils
    import concourse.bass as _bass

    if getattr(tile_utils, "max_sbuf_usage", None) is not None:
        tile_utils.max_sbuf_usage = max(tile_utils.max_sbuf_usage, 207 * 1024)

    f32 = mybir.dt.float32
    bf16 = mybir.dt.bfloat16
    f16 = mybir.dt.float16
    AF = mybir.ActivationFunctionType
    OP = mybir.AluOpType

    tok = B * Lb
    n_fc = Lb // FC               # 4 psum chunks per full-L tile
    FH = Lb // 2                  # in-proj token half
    QT = Lb // 4                  # RS quarter (tokens)
    HR = QT // N_CORES            # rows per rank per quarter

    nc = bacc.Bacc("TRN2", target_bir_lowering=False, debug=False,
                   num_devices=N_CORES)

    # ---------------- DRAM I/O ----------------
    xT = nc.dram_tensor("xT", [DM, tok], f16, kind="ExternalInput")
    w_inT = nc.dram_tensor("w_inT", [DM, 2 * E_LOC], f16, kind="ExternalInput")
    conv_diag = nc.dram_tensor("conv_diag", [128, NS * K * 128], f16,
                               kind="ExternalInput")
    conv_b = nc.dram_tensor("conv_b", [128, NS], f32, kind="ExternalInput")
    w_selT = nc.dram_tensor("w_selT", [128, NS * (R + 2 * N)], f16,
                            kind="ExternalInput")
    dt_wT = nc.dram_tensor("dt_wT", [R, E_LOC], f16, kind="ExternalInput")
    dt_b = nc.dram_tensor("dt_b", [128, NS], f32, kind="ExternalInput")
    a_cols = nc.dram_tensor("a_cols", [128, NS * N], f32, kind="ExternalInput")
    ident = nc.dram_tensor("ident", [128, 128], bf16, kind="ExternalInput")
    d_diag = nc.dram_tensor("d_diag", [128, NS * 128], f16,
                            kind="ExternalInput")
    w_outT = nc.dram_tensor("w_outT", [128, NS * DM], f16,
                            kind="ExternalInput")

    out_loc = nc.dram_tensor("out_loc", [tok // N_CORES, DM], bf16,
                             kind="ExternalOutput")

    # internal DRAM
    ar_in = [nc.dram_tensor(f"ar_in{b}", [R + 2 * N, Lb], f32)
             for b in range(B)]
    ar_out = [nc.dram_tensor(f"ar_out{b}", [R + 2 * N, Lb], f32,
                             addr_space="Shared") for b in range(B)]
    bc_sp = [nc.dram_tensor(f"bc_sp{b}", [2 * N, Lb], bf16) for b in range(B)]
    part = [nc.dram_tensor(f"part{b}", [Lb, DM], bf16) for b in range(B)]
    rs_out = [[nc.dram_tensor(f"rs_out{b}_{q}", [HR, DM], bf16)
               for q in range(4)] for b in range(B)]

    rg = [list(range(N_CORES))]

    def bcast_row(dram_t, row, width):
        """AP reading DRAM row `row` of [rows, width] broadcast to 128
        partitions."""
        sl = dram_t[row:row + 1, 0:width]
        return _bass.AP(tensor=sl.tensor, offset=sl.offset,
                        ap=[[0, 128], list(sl.ap[1])])

    with tile.TileContext(nc) as tc:
        with tc.tile_pool(name="consts", bufs=1) as consts, \
             tc.tile_pool(name="pbig", bufs=1, space="PSUM") as pbig, \
             tc.tile_pool(name="pchunk", bufs=4, space="PSUM") as pchunk, \
             tc.tile_pool(name="xt", bufs=9) as xt_pool, \
             tc.tile_pool(name="xc", bufs=2) as xc_pool, \
             tc.tile_pool(name="u", bufs=4) as u_pool, \
             tc.tile_pool(name="z", bufs=4) as z_pool, \
             tc.tile_pool(name="small", bufs=2) as small_pool, \
             tc.tile_pool(name="stage", bufs=1) as stage_pool, \
             tc.tile_pool(name="dbcp", bufs=1) as dbc_pool, \
             tc.tile_pool(name="dd", bufs=2) as dd_pool, \
             tc.tile_pool(name="rep", bufs=6) as rep_pool, \
             tc.tile_pool(name="sw", bufs=6) as sw_pool, \
             tc.tile_pool(name="y", bufs=3) as y_pool, \
             tc.tile_pool(name="gz", bufs=2) as gz_pool, \
             tc.tile_pool(name="st", bufs=5) as st_pool:

            # ---- constants ----
            w_inT_sb = consts.tile([128, DM // 128, 2 * E_LOC], f16)
            nc.sync.dma_start(out=w_inT_sb[:], in_=w_inT[:].rearrange(
                "(k p) m -> p k m", p=128))
            conv_diag_sb = consts.tile([128, NS, K, 128], f16)
            nc.sync.dma_start(out=conv_diag_sb[:], in_=conv_diag[:].rearrange(
                "p (s k m) -> p s k m", s=NS, k=K))
            conv_b_sb = consts.tile([128, NS], f32)
            nc.sync.dma_start(out=conv_b_sb[:], in_=conv_b[:])
            w_selT_sb = consts.tile([128, NS, R + 2 * N], f16)
            nc.sync.dma_start(out=w_selT_sb[:], in_=w_selT[:].rearrange(
                "p (s m) -> p s m", s=NS))
            dt_wT_sb = consts.tile([R, E_LOC], f16)
            nc.sync.dma_start(out=dt_wT_sb[:], in_=dt_wT[:])
            dt_b_sb = consts.tile([128, NS], f32)
            nc.sync.dma_start(out=dt_b_sb[:], in_=dt_b[:])
            a_cols_sb = consts.tile([128, NS * N], f32)
            nc.sync.dma_start(out=a_cols_sb[:], in_=a_cols[:])
            ident_sb = consts.tile([128, 128], bf16)
            nc.sync.dma_start(out=ident_sb[:], in_=ident[:])
            d_diag_sb = consts.tile([128, NS, 128], f16)
            nc.sync.dma_start(out=d_diag_sb[:], in_=d_diag[:].rearrange(
                "p (s m) -> p s m", s=NS))
            w_outT_sb = consts.tile([128, NS, DM], f16)
            nc.sync.dma_start(out=w_outT_sb[:], in_=w_outT[:].rearrange(
                "p (s m) -> p s m", s=NS))

            u_tiles = {}
            z_tiles = {}
            y_tiles = {}

            # ================= phase 1 (per batch) =================
            def phase1(b):
                """in-proj, conv, dbc, AllReduce, z for batch b (generator:
                yields between emission chunks for interleaving)."""
                xc_tiles = {s: xc_pool.tile([128, Lb], f16, tag="xc",
                                            name=f"xc_{b}_{s}")
                            for s in range(NS)}
                for s in range(NS):
                    u_tiles[(b, s)] = u_pool.tile([128, Lb], f16, tag="u",
                                                  name=f"u_{b}_{s}")
                    z_tiles[(b, s)] = z_pool.tile([128, Lb], bf16, tag="z",
                                                  name=f"z_{b}_{s}")
                for fh in range(2):
                    xt_sb = []
                    for k in range(DM // 128):
                        t = xt_pool.tile([128, FH], f16, tag="xt")
                        nc.sync.dma_start(
                            out=t[:],
                            in_=xT[k * 128:(k + 1) * 128,
                                   b * Lb + fh * FH:b * Lb + (fh + 1) * FH])
                        xt_sb.append(t)
                    yield
                    for m in range(4):
                        s = m % 2
                        for c in range(FH // FC):
                            pc = pchunk.tile([128, FC], f32, tag="pc",
                                             name=f"pin_{b}_{fh}_{m}_{c}")
                            for k in range(DM // 128):
                                nc.tensor.matmul(
                                    pc[:],
                                    lhsT=w_inT_sb[:, k, m * 128:(m + 1) * 128],
                                    rhs=xt_sb[k][:, c * FC:(c + 1) * FC],
                                    start=(k == 0), stop=(k == DM // 128 - 1))
                            off = fh * FH + c * FC
                            if m < 2:
                                nc.scalar.copy(
                                    xc_tiles[s][:, off:off + FC], pc[:])
                            else:
                                nc.scalar.copy(
                                    z_tiles[(b, s)][:, off:off + FC], pc[:])
                        yield
                # conv: 3 diag-matmul taps + fused silu
                for s in range(NS):
                    xc = xc_tiles[s]
                    for c in range(n_fc):
                        lo = c * FC
                        pcv = pchunk.tile([128, FC], f32, tag="pc",
                                          name=f"pcv_{b}_{s}_{c}")
                        nc.tensor.matmul(
                            pcv[:], lhsT=conv_diag_sb[:, s, 2, :],
                            rhs=xc[:, lo:lo + FC], start=True, stop=False)
                        e1 = 1 if c == 0 else 0
                        nc.tensor.matmul(
                            pcv[:, e1:FC], lhsT=conv_diag_sb[:, s, 1, :],
                            rhs=xc[:, lo + e1 - 1:lo + FC - 1],
                            start=False, stop=False)
                        e2 = 2 if c == 0 else 0
                        nc.tensor.matmul(
                            pcv[:, e2:FC], lhsT=conv_diag_sb[:, s, 0, :],
                            rhs=xc[:, lo + e2 - 2:lo + FC - 2],
                            start=False, stop=True)
                        sg = st_pool.tile([128, FC], f32, tag="st",
                                          name=f"sgc_{b}_{s}_{c}")
                        nc.scalar.activation(sg[:], pcv[:], AF.Sigmoid,
                                             bias=conv_b_sb[:, s:s + 1])
                        nc.vector.scalar_tensor_tensor(
                            u_tiles[(b, s)][:, lo:lo + FC], pcv[:],
                            conv_b_sb[:, s:s + 1], sg[:],
                            op0=OP.add, op1=OP.mult)
                        yield
                # dbc partials + AllReduce (whole batch in one call)
                dbc_sb = dbc_pool.tile([R + 2 * N, Lb], f32, tag="dbc",
                                       name=f"dbc_{b}")
                for c in range(n_fc):
                    pd = pchunk.tile([R + 2 * N, FC], f32, tag="pc",
                                     name=f"pdbc_{b}_{c}")
                    for s in range(NS):
                        nc.tensor.matmul(
                            pd[:], lhsT=w_selT_sb[:, s, :],
                            rhs=u_tiles[(b, s)][:, c * FC:(c + 1) * FC],
                            start=(s == 0), stop=(s == NS - 1))
                    nc.scalar.copy(dbc_sb[:, c * FC:(c + 1) * FC], pd[:])
                    yield
                nc.sync.dma_start(out=ar_in[b][:], in_=dbc_sb[:])
                nc.gpsimd.collective_compute(
                    "AllReduce", OP.add, replica_groups=rg,
                    ins=[ar_in[b][:]], outs=[ar_out[b][:]])
                yield

            # ================= phase 2 =================
            def prep_batch(b):
                """Stage AllReduce output; dtlow fp16 + B/C rows to DRAM."""
                stage = stage_pool.tile([R + 2 * N, Lb], f32, tag="stage",
                                        name=f"stage_{b}")
                nc.sync.dma_start(out=stage[:], in_=ar_out[b][:])
                dtlow = dd_pool.tile([R, Lb], f16, tag="dtlow",
                                     name=f"dtlow_{b}")
                nc.scalar.copy(dtlow[:], stage[0:R, :])
                btct = small_pool.tile([2 * N, Lb], bf16, tag="btct",
                                       name=f"btct_{b}")
                nc.vector.tensor_copy(btct[:], stage[R:R + 2 * N, :])
                nc.sync.dma_start(out=bc_sp[b][:], in_=btct[:])
                return dtlow

            def prep_s(b, s, dtlow):
                """delta (softplus) and du for (b, s)."""
                delta = dd_pool.tile([128, Lb], f16, tag="delta",
                                     name=f"delta_{b}_{s}")
                for c in range(n_fc):
                    pd = pchunk.tile([128, FC], f32, tag="pc",
                                     name=f"pdt_{b}_{s}_{c}")
                    nc.tensor.matmul(
                        pd[:], lhsT=dt_wT_sb[:, s * 128:(s + 1) * 128],
                        rhs=dtlow[:, c * FC:(c + 1) * FC],
                        start=True, stop=True)
                    et = st_pool.tile([128, FC], f32, tag="st",
                                      name=f"et_{b}_{s}_{c}")
                    nc.scalar.activation(et[:], pd[:], AF.Exp,
                                         bias=dt_b_sb[:, s:s + 1])
                    nc.scalar.activation(delta[:, c * FC:(c + 1) * FC],
                                         et[:], AF.Ln, bias=1.0)
                du = dd_pool.tile([128, Lb], f16, tag="du",
                                  name=f"du_{b}_{s}")
                nc.vector.tensor_mul(du[:], delta[:], u_tiles[(b, s)][:])
                return delta, du

            def scan_s(b, s, delta, du, bg=None, bg_steps=1):
                """16-state scan for (b, s); returns the open PSUM y tile.
                bg: generator to step between n-iterations (interleave)."""
                py = pbig.tile([128, Lb], f32, tag="pbig", name=f"py_{b}_{s}")
                reps = {}

                def fetch(n):
                    br = rep_pool.tile([128, Lb], bf16, tag="rep",
                                       name=f"br_{b}_{s}_{n}")
                    nc.sync.dma_start(out=br[:], in_=bcast_row(bc_sp[b], n, Lb))
                    cr = rep_pool.tile([128, Lb], bf16, tag="rep",
                                       name=f"cr_{b}_{s}_{n}")
                    nc.sync.dma_start(out=cr[:],
                                      in_=bcast_row(bc_sp[b], N + n, Lb))
                    reps[n] = (br, cr)

                fetch(0)
                fetch(1)
                for n in range(N):
                    if n + 2 < N:
                        fetch(n + 2)
                    br, cr = reps.pop(n)
                    a_sb = sw_pool.tile([128, Lb], bf16, tag="sw",
                                        name=f"a_{b}_{s}_{n}")
                    nc.scalar.activation(
                        a_sb[:], delta[:], AF.Exp,
                        scale=a_cols_sb[:, s * N + n:s * N + n + 1])
                    b_sb = sw_pool.tile([128, Lb], bf16, tag="sw",
                                        name=f"b_{b}_{s}_{n}")
                    beng = nc.gpsimd if n in GPS_BM else nc.vector
                    beng.tensor_mul(b_sb[:], du[:], br[:])
                    h_sb = sw_pool.tile([128, Lb], bf16, tag="sw",
                                        name=f"h_{b}_{s}_{n}")
                    nc.vector.tensor_tensor_scan(
                        h_sb[:], a_sb[:], b_sb[:], 0.0,
                        op0=OP.mult, op1=OP.add)
                    hc_sb = sw_pool.tile([128, Lb], bf16, tag="sw",
                                         name=f"hc_{b}_{s}_{n}")
                    heng = nc.gpsimd if n in GPS_HC else nc.vector
                    heng.tensor_mul(hc_sb[:], h_sb[:], cr[:])
                    for c in range(n_fc):
                        nc.tensor.matmul(
                            py[:, c * FC:(c + 1) * FC], lhsT=ident_sb[:],
                            rhs=hc_sb[:, c * FC:(c + 1) * FC],
                            start=(n == 0), stop=False)
                    if bg is not None:
                        for _ in range(bg_steps):
                            next(bg, None)
                # skip term: py += diag(D) @ u
                for c in range(n_fc):
                    nc.tensor.matmul(
                        py[:, c * FC:(c + 1) * FC], lhsT=d_diag_sb[:, s, :],
                        rhs=u_tiles[(b, s)][:, c * FC:(c + 1) * FC],
                        start=False, stop=True)
                return py

            def yasm(b, s, py):
                """Drain PSUM y through silu(z) gating into fp16 SBUF."""
                z = z_tiles[(b, s)]
                sg = gz_pool.tile([128, Lb], bf16, tag="sg",
                                  name=f"sgz_{b}_{s}")
                nc.scalar.activation(sg[:], z[:], AF.Sigmoid)
                yg1 = gz_pool.tile([128, Lb], bf16, tag="yg1",
                                   name=f"yg1_{b}_{s}")
                nc.vector.tensor_mul(yg1[:], py[:], z[:])
                yg = y_pool.tile([128, Lb], f16, tag="y", name=f"yg_{b}_{s}")
                nc.vector.tensor_mul(yg[:], yg1[:], sg[:])
                y_tiles[(b, s)] = yg

            def outproj(b):
                """Row-parallel out-proj with quarter ReduceScatters
                (generator)."""
                for mt in range(Lb // 128):
                    for f in range(DM // FC):
                        po = pchunk.tile([128, FC], f32, tag="pc",
                                         name=f"po_{b}_{mt}_{f}")
                        for s in range(NS):
                            nc.tensor.matmul(
                                po[:],
                                lhsT=y_tiles[(b, s)][:, mt * 128:(mt + 1) * 128],
                                rhs=w_outT_sb[:, s, f * FC:(f + 1) * FC],
                                start=(s == 0), stop=(s == NS - 1))
                        sto = st_pool.tile([128, FC], bf16, tag="st",
                                           name=f"sto_{b}_{mt}_{f}")
                        if (mt + f) % 2 == 0:
                            nc.scalar.copy(sto[:], po[:])
                        else:
                            nc.vector.tensor_copy(sto[:], po[:])
                        nc.sync.dma_start(
                            out=part[b][mt * 128:(mt + 1) * 128,
                                        f * FC:(f + 1) * FC],
                            in_=sto[:])
                        yield
                    if mt % 4 == 3:
                        q = mt // 4
                        nc.gpsimd.collective_compute(
                            "ReduceScatter", OP.add, replica_groups=rg,
                            ins=[part[b][q * QT:(q + 1) * QT, :]],
                            outs=[rs_out[b][q][:]])
                        nc.gpsimd.dma_start(
                            out=out_loc[(b * 4 + q) * HR:
                                        (b * 4 + q + 1) * HR, :],
                            in_=rs_out[b][q][:])
                        yield

            def run_gen(g):
                for _ in g:
                    pass

            # --------- emission schedule ---------
            g_p1_0 = phase1(0)
            run_gen(g_p1_0)                      # batch 0 phase 1 + AR
            g_p1_1 = phase1(1)                   # batch 1 phase 1 interleaved
            for _ in range(10):                  # keep PE busy during AR(b0)
                next(g_p1_1, None)
            dtlow0 = prep_batch(0)
            d0, du0 = prep_s(0, 0, dtlow0)
            py = scan_s(0, 0, d0, du0, bg=g_p1_1)
            yasm(0, 0, py)
            d1, du1 = prep_s(0, 1, dtlow0)
            py = scan_s(0, 1, d1, du1, bg=g_p1_1)
            run_gen(g_p1_1)
            yasm(0, 1, py)
            dtlow1 = prep_batch(1)
            d2, du2 = prep_s(1, 0, dtlow1)
            g_op0 = outproj(0)
            py = scan_s(1, 0, d2, du2, bg=g_op0)
            yasm(1, 0, py)
            d3, du3 = prep_s(1, 1, dtlow1)
            py = scan_s(1, 1, d3, du3, bg=g_op0)
            run_gen(g_op0)
            yasm(1, 1, py)
            g_op1 = outproj(1)
            run_gen(g_op1)

    nc.compile()
    _PROGRAM_CACHE[key] = nc
    return nc


# ---------------------------------------------------------------------------
def host_prep(inputs, Lb=L):
    x = np.asarray(inputs["x"], np.float32)
    W_in = np.asarray(inputs["W_in"], np.float32)
    conv_w = np.asarray(inputs["conv_w"], np.float32)
    conv_b = np.asarray(inputs["conv_b"], np.float32)
    W_sel = np.asarray(inputs["W_sel"], np.float32)
    dt_w = np.asarray(inputs["dt_w"], np.float32)
    dt_b = np.asarray(inputs["dt_b"], np.float32)
    A_log = np.asarray(inputs["A_log"], np.float32)
    D_param = np.asarray(inputs["D_param"], np.float32)
    W_out = np.asarray(inputs["W_out"], np.float32)

    import ml_dtypes
    bf16 = ml_dtypes.bfloat16
    tok = B * Lb
    xT = np.ascontiguousarray(
        x[:, :Lb, :].reshape(tok, DM).T).astype(np.float16)
    A = -np.exp(A_log.astype(np.float64)).astype(np.float32)   # [E, N]

    ident = np.eye(128, dtype=np.float32)

    in_maps = []
    for k in range(N_CORES):
        es = slice(k * E_LOC, (k + 1) * E_LOC)
        W_in_loc = np.concatenate([W_in[k * E_LOC:(k + 1) * E_LOC],
                                   W_in[E + k * E_LOC:E + (k + 1) * E_LOC]],
                                  axis=0)            # [2*E_LOC, DM]
        A_loc = A[es]                                # [E_LOC, N]

        # a_cols[p, s*N + n] = A_loc[s*128+p, n]
        a_cols = np.zeros((128, NS * N), np.float32)
        for s in range(NS):
            for n in range(N):
                a_cols[:, s * N + n] = A_loc[s * 128:(s + 1) * 128, n]

        # conv_diag[p, s, kk, :] = diag of conv_w[es][s*128+p] tap kk
        conv_diag = np.zeros((128, NS, K, 128), np.float32)
        for s in range(NS):
            for kk in range(K):
                conv_diag[:, s, kk, :] = np.diag(
                    conv_w[es][s * 128:(s + 1) * 128, 0, kk])
        d_diag = np.zeros((128, NS, 128), np.float32)
        for s in range(NS):
            d_diag[:, s, :] = np.diag(D_param[es][s * 128:(s + 1) * 128])

        def two(v):  # [E_LOC] -> [128, NS]
            return np.ascontiguousarray(v.reshape(NS, 128).T)

        in_maps.append({
            "xT": xT,
            "w_inT": np.ascontiguousarray(W_in_loc.T).astype(np.float16),
            "conv_diag": np.ascontiguousarray(
                conv_diag.reshape(128, NS * K * 128)).astype(np.float16),
            "conv_b": two(conv_b[es]),
            "w_selT": np.ascontiguousarray(
                W_sel[:, es].T.reshape(NS, 128, R + 2 * N).transpose(
                    1, 0, 2).reshape(128, NS * (R + 2 * N))).astype(
                        np.float16),
            "dt_wT": np.ascontiguousarray(dt_w[es].T).astype(np.float16),
            "dt_b": two(dt_b[es]),
            "a_cols": a_cols,
            "ident": ident.astype(bf16),
            "d_diag": np.ascontiguousarray(
                d_diag.reshape(128, NS * 128)).astype(np.float16),
            "w_outT": np.ascontiguousarray(
                W_out[:, es].T.reshape(NS, 128, DM).transpose(
                    1, 0, 2).reshape(128, NS * DM)).astype(np.float16),
        })
    return in_maps


def assemble_output(results, Lb=L):
    out = np.empty((B, Lb, DM), np.float32)
    QT = Lb // 4
    hr = QT // N_CORES
    for c in range(N_CORES):
        chunk = np.asarray(results[c]["out_loc"], np.float32)
        for b in range(B):
            for q in range(4):
                out[b, q * QT + c * hr:q * QT + (c + 1) * hr, :] = \
                    chunk[(b * 4 + q) * hr:(b * 4 + q + 1) * hr, :]
    return out


def kernel(**inputs) -> np.ndarray:
    from concourse import bass_utils
    nc = build_program()
    in_maps = host_prep(inputs)
    res = bass_utils.run_bass_kernel_spmd(nc, in_maps, list(range(N_CORES)))
    return assemble_output(res.results).astype(np.float32)
